# revision 25
# baseline (speedup 1.0000x reference)
"""Distributed Trainium2 kernel for GPT-2 attention block with LoRA on Q/V.

Sharding: 8 cores = 2 batches x 4 head-groups (4 heads each).
Each core computes QKV (its head slice), causal attention for its 4 heads,
and a partial c_proj contraction over its head features. Host sums the 4
partials per batch (the unshard step for a contraction-sharded output) and
reassembles k/v into `present`.

All GEMMs run in bf16 with fp32 PSUM accumulation. LoRA is folded into the
Q/V weight columns on the host (exact algebra: W_eff = W + scaling*A^T B^T).
"""

import numpy as np
import ml_dtypes
from contextlib import ExitStack

import concourse.bass as bass
import concourse.tile as tile
from concourse import bacc, mybir
from concourse.bass_utils import run_bass_kernel_spmd

# Problem constants (hardcoded; kernel.py must be self-contained)
B, S, NX, NH, HD = 2, 2048, 1024, 16, 64
R, SCALING = 4, 8.0
NCORES = 8
HPC = 4            # heads per core
CPW = HPC * HD     # 256 head-feature columns per core
P = 128
ST = 512           # free-dim tile for matmuls
NSQT = S // ST     # 4
NSC = S // P       # 16
DC = NX // P       # 8 contraction chunks

BF = mybir.dt.bfloat16
F32 = mybir.dt.float32
EXP = mybir.ActivationFunctionType.Exp

# vaug column layout per head pair (193 cols):
#   [v_even(64) | ones(1) | z(32) | ones(1) | z(31) | v_odd(64)]
# odd head lhsT = cols 65..193 of the pair block: sumexp lands on out row 32.
VAUG_PAIR = 193
VAUG_W = 2 * VAUG_PAIR  # 386
V_COL_START = [0, 129, 193, 322]   # v columns for local heads 0..3
ONES_COLS = [64, 97, 257, 290]

_CACHED = {}


def _build_graph():
    nc = bacc.Bacc("TRN2", target_bir_lowering=False, debug=False, num_devices=NCORES)

    xt_d = nc.dram_tensor("xt", [NSC, DC, P, P], BF, kind="ExternalInput").ap()
    wqk_d = nc.dram_tensor("wqk", [P, DC, 2 * CPW], BF, kind="ExternalInput").ap()
    wv_d = nc.dram_tensor("wv", [P, DC, CPW], BF, kind="ExternalInput").ap()
    wp_d = nc.dram_tensor("wp", [P, 2, NX], BF, kind="ExternalInput").ap()
    bqk_d = nc.dram_tensor("bqk", [P, 4], F32, kind="ExternalInput").ap()
    bv_d = nc.dram_tensor("bv", [1, CPW], BF, kind="ExternalInput").ap()
    msk_d = nc.dram_tensor("msk", [P, 4, ST], BF, kind="ExternalInput").ap()

    kt_d = nc.dram_tensor("kt_out", [2, P, S], BF, kind="ExternalOutput").ap()
    v_d = nc.dram_tensor("v_out", [NSC, P, VAUG_W], BF, kind="ExternalOutput").ap()
    part_d = nc.dram_tensor("part", [NSC, P, NX], F32, kind="ExternalOutput").ap()

    with tile.TileContext(nc) as tc:
        with ExitStack() as ctx:
            _emit(ctx, tc, nc, xt_d, wqk_d, wv_d, wp_d, bqk_d, bv_d, msk_d,
                  kt_d, v_d, part_d)

    nc.compile()
    return nc


def _emit(ctx, tc, nc, xt_d, wqk_d, wv_d, wp_d, bqk_d, bv_d, msk_d,
          kt_d, v_d, part_d):
    const = ctx.enter_context(tc.tile_pool(name="const", bufs=1))
    work = ctx.enter_context(tc.tile_pool(name="work", bufs=6))
    stage = ctx.enter_context(tc.tile_pool(name="stage", bufs=4))
    psmm = ctx.enter_context(tc.tile_pool(name="psmm", bufs=5, space="PSUM"))
    psav = ctx.enter_context(tc.tile_pool(name="psav", bufs=3, space="PSUM"))

    xt_sb = const.tile([P, DC, S], BF, tag="xt")
    wqk_sb = const.tile([P, DC, 2 * CPW], BF, tag="wqk")
    wv_sb = const.tile([P, DC, CPW], BF, tag="wv")
    wp_sb = const.tile([P, 2, NX], BF, tag="wp")
    bqk_sb = const.tile([P, 4], F32, tag="bqk")
    bv_sb = const.tile([1, CPW], BF, tag="bv")
    msk_sb = const.tile([P, 4, ST], BF, tag="msk")
    ones_sb = const.tile([1, P], BF, tag="ones")
    qt_sb = [const.tile([P, S], BF, tag=f"qt{c}", name=f"qt{c}") for c in range(2)]
    kt_sb = [const.tile([P, S], BF, tag=f"kt{c}", name=f"kt{c}") for c in range(2)]
    vaug_sb = const.tile([P, NSC, VAUG_W], BF, tag="vaug")
    act_sb = [const.tile([P, S], BF, tag=f"act{c}", name=f"act{c}") for c in range(2)]

    # input DMAs: weights first, then xt in column-quarters so the first
    # GEMM s-tile can start after ~1/4 of x has landed
    nc.sync.dma_start(wv_sb[:], wv_d)
    for q in range(4):
        nc.sync.dma_start(xt_sb[:, :, q * P:(q + 1) * P],
                          xt_d[q].rearrange("o p s -> p o s"))
    nc.sync.dma_start(wqk_sb[:], wqk_d)
    nc.sync.dma_start(bqk_sb[:], bqk_d)
    nc.sync.dma_start(bv_sb[:], bv_d)
    nc.sync.dma_start(msk_sb[:], msk_d)
    for q in range(4, NSC):
        nc.sync.dma_start(xt_sb[:, :, q * P:(q + 1) * P],
                          xt_d[q].rearrange("o p s -> p o s"))
    nc.sync.dma_start(wp_sb[:], wp_d)

    nc.gpsimd.memset(ones_sb[:], 1.0)
    for z0, z1 in ((65, 129), (258, 322)):  # zero pads of the odd-head lhsT
        nc.vector.memset(vaug_sb[:, :, z0:z1], 0.0)
    for col in ONES_COLS:
        nc.vector.memset(vaug_sb[:, :, col:col + 1], 1.0)

    # ---- unit-based emission: attention quarters interleave with the next
    # quarter's GEMM units so PE has work while ACT chews the exp backlog.
    def qk_units(t):
        def one(jc):
            def u():
                ps = psmm.tile([P, ST], F32, tag="psmm", name=f"qkps{t}{jc}")
                for dc in range(DC):
                    nc.tensor.matmul(
                        ps, lhsT=wqk_sb[:, dc, jc * P:(jc + 1) * P],
                        rhs=xt_sb[:, dc, t * ST:(t + 1) * ST],
                        start=(dc == 0), stop=(dc == DC - 1))
                dst = (qt_sb if jc < 2 else kt_sb)[jc % 2]
                nc.vector.tensor_scalar_add(
                    dst[:, t * ST:(t + 1) * ST], ps, bqk_sb[:, jc:jc + 1])
            return u
        return [one(jc) for jc in range(4)]

    def v_units(t):
        def one(i):
            def u():
                ps = psmm.tile([P, ST], F32, tag="psmm", name=f"vps{i}")
                psv = ps[:, :CPW]
                for dc in range(DC):
                    nc.tensor.matmul(
                        psv, lhsT=xt_sb[:, dc, i * P:(i + 1) * P],
                        rhs=wv_sb[:, dc, :], start=(dc == 0), stop=False)
                nc.tensor.matmul(psv, lhsT=ones_sb[0:1, :], rhs=bv_sb[:],
                                 start=False, stop=True)
                vdst = vaug_sb[:, i, :].rearrange("p (b c) -> p b c", c=VAUG_PAIR)
                vsrc = psv.rearrange("p (b c) -> p b c", c=P)
                nc.vector.tensor_copy(vdst[:, :, 0:64], vsrc[:, :, 0:64])
                nc.vector.tensor_copy(vdst[:, :, 129:193], vsrc[:, :, 64:128])
            return u
        return [one(i) for i in range(4 * t, 4 * t + 4)]

    def attn_units(j):
        n_i = 4 * j + 4  # causal: s_k chunks 0..4j+3
        units = []
        for pp in range(2):
            state = {}

            def alloc(pp=pp, state=state):
                state["av0"] = psav.tile([P, ST], F32, tag="psav",
                                         name=f"av0_{j}{pp}")
                state["av1"] = psav.tile([P, ST], F32, tag="psav",
                                         name=f"av1_{j}{pp}")

            def iter_unit(i, pp=pp, state=state):
                def u():
                    c = pp
                    av0, av1 = state["av0"], state["av1"]
                    d = i - 4 * j
                    c0 = 0 if d < 0 else P * d  # first causally-live col
                    qs = slice(j * ST + c0, (j + 1) * ST)
                    qk0 = psmm.tile([P, ST], F32, tag="psmm",
                                    name=f"qk0_{j}{pp}{i}")
                    qk1 = psmm.tile([P, ST], F32, tag="psmm",
                                    name=f"qk1_{j}{pp}{i}")
                    nc.tensor.matmul(
                        qk0[:, c0:], lhsT=kt_sb[c][0:64, i * P:(i + 1) * P],
                        rhs=qt_sb[c][0:64, qs], start=True, stop=True)
                    nc.tensor.matmul(
                        qk1[:, c0:], lhsT=kt_sb[c][64:128, i * P:(i + 1) * P],
                        rhs=qt_sb[c][64:128, qs], start=True, stop=True)
                    e0 = work.tile([P, ST], BF, tag="exp", name=f"e0_{j}{pp}{i}")
                    e1 = work.tile([P, ST], BF, tag="exp", name=f"e1_{j}{pp}{i}")
                    for e, qk in ((e0, qk0), (e1, qk1)):
                        nc.scalar.activation(e[:, c0:], qk[:, c0:], EXP,
                                             scale=0.125)
                        if d >= 0:
                            # triangular 128-col band at the causal edge
                            nc.vector.tensor_mul(
                                e[:, c0:c0 + P], e[:, c0:c0 + P],
                                msk_sb[:, d, c0:c0 + P])
                    vb = vaug_sb[:, i, :]
                    first, last = (i == 0), (i == n_i - 1)
                    nc.tensor.matmul(
                        av0[0:65, c0:],
                        lhsT=vb[:, pp * VAUG_PAIR:pp * VAUG_PAIR + 65],
                        rhs=e0[:, c0:], start=first, stop=last)
                    nc.tensor.matmul(
                        av1[:, c0:],
                        lhsT=vb[:, pp * VAUG_PAIR + 65:(pp + 1) * VAUG_PAIR],
                        rhs=e1[:, c0:], start=first, stop=last)
                return u

            def norm(pp=pp, state=state):
                # normalize: rows /= sumexp row (64 for even head, 32 for odd)
                c = pp
                for av, sumrow, lo, hi in ((state["av0"], 64, 0, 64),
                                           (state["av1"], 32, 64, 128)):
                    # partition-shifted reciprocal lands the row on partition
                    # 0, where HW partition_broadcast reads its source
                    rec = work.tile([1, ST], F32, tag="rec")
                    nc.vector.reciprocal(rec[0:1, :],
                                         av[sumrow:sumrow + 1, :])
                    bcast = work.tile([P, ST], F32, tag="bcast")
                    nc.gpsimd.partition_broadcast(bcast[:], rec[:])
                    nc.vector.tensor_mul(
                        act_sb[c][lo:hi, j * ST:(j + 1) * ST],
                        av[lo:hi, :], bcast[lo:hi, :])

            units.append(alloc)
            units.extend(iter_unit(i) for i in range(n_i))
            units.append(norm)
        return units

    def proj_units(q):
        # partial c_proj: part[s, n] = sum_j actT[j, s] * Wp[j, n]
        def one(t):
            def u():
                st = stage.tile([P, NX], F32, tag="stage", name=f"pst{t}")
                for nh in range(2):
                    ps = psmm.tile([P, ST], F32, tag="psmm", name=f"pps{t}{nh}")
                    for c in range(2):
                        nc.tensor.matmul(
                            ps, lhsT=act_sb[c][:, t * P:(t + 1) * P],
                            rhs=wp_sb[:, c, nh * ST:(nh + 1) * ST],
                            start=(c == 0), stop=(c == 1))
                    nc.vector.tensor_copy(st[:, nh * ST:(nh + 1) * ST], ps[:])
                nc.sync.dma_start(part_d[t], st[:])
            return u
        return [one(t) for t in range(4 * q, 4 * q + 4)]

    def run_interleaved(primary, filler):
        if not filler:
            for u in primary:
                u()
            return
        step = len(primary) / (len(filler) + 1)
        fi = 0
        for k, u in enumerate(primary):
            u()
            while fi < len(filler) and (fi + 1) * step <= k + 1:
                filler[fi]()
                fi += 1
        for u in filler[fi:]:
            u()

    for u in v_units(0) + qk_units(0):
        u()
    for t in range(NSQT):
        filler = []
        if t + 1 < NSQT:
            filler += qk_units(t + 1) + v_units(t + 1)
        if t >= 1:
            filler += proj_units(t - 1)
        run_interleaved(attn_units(t), filler)
        nc.sync.dma_start(v_d[4 * t:4 * t + 4].rearrange("t p c -> p t c"),
                          vaug_sb[:, 4 * t:4 * t + 4, :])
        for c in range(2):
            nc.sync.dma_start(kt_d[c, :, t * ST:(t + 1) * ST],
                              kt_sb[c][:, t * ST:(t + 1) * ST])
    for u in proj_units(NSQT - 1):
        u()


def _host_inputs(x, W_attn, b_attn, lora_A, lora_B, W_proj, b_proj):
    """Build the 8 per-core input maps (all numpy, bf16 for GEMM operands)."""
    bf = ml_dtypes.bfloat16
    x = np.asarray(x, np.float32)
    W_attn = np.asarray(W_attn, np.float32)
    b_attn = np.asarray(b_attn, np.float32)
    lora_A = np.asarray(lora_A, np.float32)
    lora_B = np.asarray(lora_B, np.float32)
    W_proj = np.asarray(W_proj, np.float32)

    Wq = W_attn[:, :NX] + SCALING * (lora_A[:R].T @ lora_B[:NX].T)
    Wk = W_attn[:, NX:2 * NX]
    Wv = W_attn[:, 2 * NX:] + SCALING * (lora_A[R:].T @ lora_B[NX:].T)

    # causal diagonal masks: keep when f >= 128*d + p
    f = np.arange(ST)[None, :]
    p = np.arange(P)[:, None]
    msk = np.ascontiguousarray(np.stack(
        [(f >= (P * d + p)) for d in range(4)]).transpose(1, 0, 2)).astype(bf)

    in_maps = []
    for core in range(NCORES):
        b, g = core // 4, core % 4
        cs = CPW * g
        xt = np.ascontiguousarray(
            x[b].T.reshape(DC, P, NSC, P).transpose(2, 0, 1, 3)).astype(bf)
        wqk = np.concatenate([Wq[:, cs:cs + CPW], Wk[:, cs:cs + CPW]], axis=1)
        wqk = np.ascontiguousarray(
            wqk.reshape(DC, P, 2 * CPW).transpose(1, 0, 2)).astype(bf)
        wv = np.ascontiguousarray(
            Wv[:, cs:cs + CPW].reshape(DC, P, CPW).transpose(1, 0, 2)).astype(bf)
        wp = np.ascontiguousarray(
            W_proj[cs:cs + CPW].reshape(2, P, NX).transpose(1, 0, 2)).astype(bf)
        bqk = np.stack([b_attn[cs:cs + P], b_attn[cs + P:cs + CPW],
                        b_attn[NX + cs:NX + cs + P],
                        b_attn[NX + cs + P:NX + cs + CPW]], axis=1)
        bqk = np.ascontiguousarray(bqk, np.float32)
        bv = b_attn[2 * NX + cs:2 * NX + cs + CPW].reshape(1, CPW).astype(bf)
        in_maps.append({"xt": xt, "wqk": wqk, "wv": wv, "wp": wp,
                        "bqk": bqk, "bv": bv, "msk": msk})
    return in_maps


def _assemble(results, b_attn, b_proj):
    b_proj = np.asarray(b_proj, np.float32)
    out = np.zeros((B, S, NX), np.float32)
    present = np.zeros((2, B, NH, S, HD), np.float32)
    for core in range(NCORES):
        b, g = core // 4, core % 4
        r = results[core]
        out[b] += r["part"].reshape(S, NX).astype(np.float32)
        kt = r["kt_out"].reshape(CPW, S).astype(np.float32)
        v = r["v_out"].reshape(S, VAUG_W).astype(np.float32)
        for hl in range(HPC):
            present[0, b, HPC * g + hl] = kt[HD * hl:HD * (hl + 1), :].T
            cs = V_COL_START[hl]
            present[1, b, HPC * g + hl] = v[:, cs:cs + HD]
    out += b_proj[None, None, :]
    return out, present


def get_graph():
    if "nc" not in _CACHED:
        _CACHED["nc"] = _build_graph()
    return _CACHED["nc"]


def kernel(x, W_attn, b_attn, lora_A, lora_B, W_proj, b_proj, **extra):
    nc = get_graph()
    in_maps = _host_inputs(x, W_attn, b_attn, lora_A, lora_B, W_proj, b_proj)
    res = run_bass_kernel_spmd(nc, in_maps, core_ids=list(range(NCORES)))
    return _assemble(res.results, b_attn, b_proj)


# revision 26
# speedup vs baseline: 1.0014x; 1.0014x over previous
"""Distributed Trainium2 kernel for GPT-2 attention block with LoRA on Q/V.

Sharding: 8 cores = 2 batches x 4 head-groups (4 heads each).
Each core computes QKV (its head slice), causal attention for its 4 heads,
and a partial c_proj contraction over its head features. Host sums the 4
partials per batch (the unshard step for a contraction-sharded output) and
reassembles k/v into `present`.

All GEMMs run in bf16 with fp32 PSUM accumulation. LoRA is folded into the
Q/V weight columns on the host (exact algebra: W_eff = W + scaling*A^T B^T).
"""

import numpy as np
import ml_dtypes
from contextlib import ExitStack

import concourse.tile as tile
from concourse import bacc, mybir
from concourse.bass_utils import run_bass_kernel_spmd

# Problem constants (hardcoded; kernel.py must be self-contained)
B, S, NX, NH, HD = 2, 2048, 1024, 16, 64
R, SCALING = 4, 8.0
NCORES = 8
HPC = 4            # heads per core
CPW = HPC * HD     # 256 head-feature columns per core
P = 128
ST = 512           # free-dim tile for matmuls
NSQT = S // ST     # 4
NSC = S // P       # 16
DC = NX // P       # 8 contraction chunks

BF = mybir.dt.bfloat16
F32 = mybir.dt.float32
EXP = mybir.ActivationFunctionType.Exp

# vaug column layout per head pair (193 cols):
#   [v_even(64) | ones(1) | z(32) | ones(1) | z(31) | v_odd(64)]
# odd head lhsT = cols 65..193 of the pair block: sumexp lands on out row 32.
VAUG_PAIR = 193
VAUG_W = 2 * VAUG_PAIR  # 386
V_COL_START = [0, 129, 193, 322]   # v columns for local heads 0..3
ONES_COLS = [64, 97, 257, 290]

_CACHED = {}


def _build_graph():
    nc = bacc.Bacc("TRN2", target_bir_lowering=False, debug=False, num_devices=NCORES)

    xt_d = nc.dram_tensor("xt", [NSC, DC, P, P], BF, kind="ExternalInput").ap()
    wqk_d = nc.dram_tensor("wqk", [P, DC, 2 * CPW], BF, kind="ExternalInput").ap()
    wv_d = nc.dram_tensor("wv", [P, DC, CPW], BF, kind="ExternalInput").ap()
    wp_d = nc.dram_tensor("wp", [P, 2, NX], BF, kind="ExternalInput").ap()
    bqk_d = nc.dram_tensor("bqk", [P, 4], F32, kind="ExternalInput").ap()
    bv_d = nc.dram_tensor("bv", [1, CPW], BF, kind="ExternalInput").ap()
    msk_d = nc.dram_tensor("msk", [P, 4, ST], BF, kind="ExternalInput").ap()

    kt_d = nc.dram_tensor("kt_out", [2, P, S], BF, kind="ExternalOutput").ap()
    v_d = nc.dram_tensor("v_out", [NSC, P, VAUG_W], BF, kind="ExternalOutput").ap()
    part_d = nc.dram_tensor("part", [NSC, P, NX], F32, kind="ExternalOutput").ap()

    with tile.TileContext(nc) as tc:
        with ExitStack() as ctx:
            _emit(ctx, tc, nc, xt_d, wqk_d, wv_d, wp_d, bqk_d, bv_d, msk_d,
                  kt_d, v_d, part_d)

    nc.compile()
    return nc


def _emit(ctx, tc, nc, xt_d, wqk_d, wv_d, wp_d, bqk_d, bv_d, msk_d,
          kt_d, v_d, part_d):
    const = ctx.enter_context(tc.tile_pool(name="const", bufs=1))
    work = ctx.enter_context(tc.tile_pool(name="work", bufs=10))
    stage = ctx.enter_context(tc.tile_pool(name="stage", bufs=4))
    psmm = ctx.enter_context(tc.tile_pool(name="psmm", bufs=5, space="PSUM"))
    psav = ctx.enter_context(tc.tile_pool(name="psav", bufs=3, space="PSUM"))

    xt_sb = const.tile([P, DC, S], BF, tag="xt")
    wqk_sb = const.tile([P, DC, 2 * CPW], BF, tag="wqk")
    wv_sb = const.tile([P, DC, CPW], BF, tag="wv")
    wp_sb = const.tile([P, 2, NX], BF, tag="wp")
    bqk_sb = const.tile([P, 4], F32, tag="bqk")
    bv_sb = const.tile([1, CPW], BF, tag="bv")
    msk_sb = const.tile([P, 4, ST], BF, tag="msk")
    ones_sb = const.tile([1, P], BF, tag="ones")
    qt_sb = [const.tile([P, S], BF, tag=f"qt{c}", name=f"qt{c}") for c in range(2)]
    kt_sb = [const.tile([P, S], BF, tag=f"kt{c}", name=f"kt{c}") for c in range(2)]
    vaug_sb = const.tile([P, NSC, VAUG_W], BF, tag="vaug")
    act_sb = [const.tile([P, S], BF, tag=f"act{c}", name=f"act{c}") for c in range(2)]

    # input DMAs: weights first, then xt in column-quarters so the first
    # GEMM s-tile can start after ~1/4 of x has landed
    nc.sync.dma_start(wv_sb[:], wv_d)
    for q in range(4):
        nc.sync.dma_start(xt_sb[:, :, q * P:(q + 1) * P],
                          xt_d[q].rearrange("o p s -> p o s"))
    nc.sync.dma_start(wqk_sb[:], wqk_d)
    nc.sync.dma_start(bqk_sb[:], bqk_d)
    nc.sync.dma_start(bv_sb[:], bv_d)
    nc.sync.dma_start(msk_sb[:], msk_d)
    for q in range(4, NSC):
        nc.sync.dma_start(xt_sb[:, :, q * P:(q + 1) * P],
                          xt_d[q].rearrange("o p s -> p o s"))
    nc.sync.dma_start(wp_sb[:], wp_d)

    nc.gpsimd.memset(ones_sb[:], 1.0)
    for z0, z1 in ((65, 129), (258, 322)):  # zero pads of the odd-head lhsT
        nc.vector.memset(vaug_sb[:, :, z0:z1], 0.0)
    for col in ONES_COLS:
        nc.vector.memset(vaug_sb[:, :, col:col + 1], 1.0)

    # ---- unit-based emission: attention quarters interleave with the next
    # quarter's GEMM units so PE has work while ACT chews the exp backlog.
    def qk_units(t):
        def one(jc):
            def u():
                ps = psmm.tile([P, ST], F32, tag="psmm", name=f"qkps{t}{jc}")
                for dc in range(DC):
                    nc.tensor.matmul(
                        ps, lhsT=wqk_sb[:, dc, jc * P:(jc + 1) * P],
                        rhs=xt_sb[:, dc, t * ST:(t + 1) * ST],
                        start=(dc == 0), stop=(dc == DC - 1))
                dst = (qt_sb if jc < 2 else kt_sb)[jc % 2]
                nc.vector.tensor_scalar_add(
                    dst[:, t * ST:(t + 1) * ST], ps, bqk_sb[:, jc:jc + 1])
            return u
        return [one(jc) for jc in range(4)]

    def v_units(t):
        def one(i):
            def u():
                ps = psmm.tile([P, ST], F32, tag="psmm", name=f"vps{i}")
                psv = ps[:, :CPW]
                for dc in range(DC):
                    nc.tensor.matmul(
                        psv, lhsT=xt_sb[:, dc, i * P:(i + 1) * P],
                        rhs=wv_sb[:, dc, :], start=(dc == 0), stop=False)
                nc.tensor.matmul(psv, lhsT=ones_sb[0:1, :], rhs=bv_sb[:],
                                 start=False, stop=True)
                vdst = vaug_sb[:, i, :].rearrange("p (b c) -> p b c", c=VAUG_PAIR)
                vsrc = psv.rearrange("p (b c) -> p b c", c=P)
                nc.vector.tensor_copy(vdst[:, :, 0:64], vsrc[:, :, 0:64])
                nc.vector.tensor_copy(vdst[:, :, 129:193], vsrc[:, :, 64:128])
            return u
        return [one(i) for i in range(4 * t, 4 * t + 4)]

    def attn_units(j):
        n_i = 4 * j + 4  # causal: s_k chunks 0..4j+3
        units = []
        for pp in range(2):
            state = {}

            def alloc(pp=pp, state=state):
                state["av0"] = psav.tile([P, ST], F32, tag="psav",
                                         name=f"av0_{j}{pp}")
                state["av1"] = psav.tile([P, ST], F32, tag="psav",
                                         name=f"av1_{j}{pp}")

            def iter_unit(i, pp=pp, state=state):
                def u():
                    c = pp
                    av0, av1 = state["av0"], state["av1"]
                    d = i - 4 * j
                    c0 = 0 if d < 0 else P * d  # first causally-live col
                    qs = slice(j * ST + c0, (j + 1) * ST)
                    qk0 = psmm.tile([P, ST], F32, tag="psmm",
                                    name=f"qk0_{j}{pp}{i}")
                    qk1 = psmm.tile([P, ST], F32, tag="psmm",
                                    name=f"qk1_{j}{pp}{i}")
                    nc.tensor.matmul(
                        qk0[:, c0:], lhsT=kt_sb[c][0:64, i * P:(i + 1) * P],
                        rhs=qt_sb[c][0:64, qs], start=True, stop=True)
                    nc.tensor.matmul(
                        qk1[:, c0:], lhsT=kt_sb[c][64:128, i * P:(i + 1) * P],
                        rhs=qt_sb[c][64:128, qs], start=True, stop=True)
                    e0 = work.tile([P, ST], BF, tag="exp", name=f"e0_{j}{pp}{i}")
                    e1 = work.tile([P, ST], BF, tag="exp", name=f"e1_{j}{pp}{i}")
                    for e, qk in ((e0, qk0), (e1, qk1)):
                        nc.scalar.activation(e[:, c0:], qk[:, c0:], EXP,
                                             scale=0.125)
                        if d >= 0:
                            # triangular 128-col band at the causal edge
                            nc.vector.tensor_mul(
                                e[:, c0:c0 + P], e[:, c0:c0 + P],
                                msk_sb[:, d, c0:c0 + P])
                    vb = vaug_sb[:, i, :]
                    first, last = (i == 0), (i == n_i - 1)
                    nc.tensor.matmul(
                        av0[0:65, c0:],
                        lhsT=vb[:, pp * VAUG_PAIR:pp * VAUG_PAIR + 65],
                        rhs=e0[:, c0:], start=first, stop=last)
                    nc.tensor.matmul(
                        av1[:, c0:],
                        lhsT=vb[:, pp * VAUG_PAIR + 65:(pp + 1) * VAUG_PAIR],
                        rhs=e1[:, c0:], start=first, stop=last)
                return u

            def norm(pp=pp, state=state):
                # normalize: rows /= sumexp row (64 for even head, 32 for odd)
                c = pp
                for av, sumrow, lo, hi in ((state["av0"], 64, 0, 64),
                                           (state["av1"], 32, 64, 128)):
                    # partition-shifted reciprocal lands the row on partition
                    # 0, where HW partition_broadcast reads its source
                    rec = work.tile([1, ST], F32, tag="rec")
                    nc.vector.reciprocal(rec[0:1, :],
                                         av[sumrow:sumrow + 1, :])
                    bcast = work.tile([P, ST], F32, tag="bcast")
                    nc.gpsimd.partition_broadcast(bcast[:], rec[:])
                    nc.vector.tensor_mul(
                        act_sb[c][lo:hi, j * ST:(j + 1) * ST],
                        av[lo:hi, :], bcast[lo:hi, :])

            units.append(alloc)
            units.extend(iter_unit(i) for i in range(n_i))
            units.append(norm)
        return units

    def proj_units(q):
        # partial c_proj: part[s, n] = sum_j actT[j, s] * Wp[j, n]
        def one(t):
            def u():
                st = stage.tile([P, NX], F32, tag="stage", name=f"pst{t}")
                for nh in range(2):
                    ps = psmm.tile([P, ST], F32, tag="psmm", name=f"pps{t}{nh}")
                    for c in range(2):
                        nc.tensor.matmul(
                            ps, lhsT=act_sb[c][:, t * P:(t + 1) * P],
                            rhs=wp_sb[:, c, nh * ST:(nh + 1) * ST],
                            start=(c == 0), stop=(c == 1))
                    nc.vector.tensor_copy(st[:, nh * ST:(nh + 1) * ST], ps[:])
                nc.sync.dma_start(part_d[t], st[:])
            return u
        return [one(t) for t in range(4 * q, 4 * q + 4)]

    def run_interleaved(primary, filler):
        if not filler:
            for u in primary:
                u()
            return
        step = len(primary) / (len(filler) + 1)
        fi = 0
        for k, u in enumerate(primary):
            u()
            while fi < len(filler) and (fi + 1) * step <= k + 1:
                filler[fi]()
                fi += 1
        for u in filler[fi:]:
            u()

    for u in v_units(0) + qk_units(0):
        u()
    for t in range(NSQT):
        filler = []
        if t + 1 < NSQT:
            filler += qk_units(t + 1) + v_units(t + 1)
        if t >= 1:
            filler += proj_units(t - 1)
        run_interleaved(attn_units(t), filler)
        nc.sync.dma_start(v_d[4 * t:4 * t + 4].rearrange("t p c -> p t c"),
                          vaug_sb[:, 4 * t:4 * t + 4, :])
        for c in range(2):
            nc.sync.dma_start(kt_d[c, :, t * ST:(t + 1) * ST],
                              kt_sb[c][:, t * ST:(t + 1) * ST])
    for u in proj_units(NSQT - 1):
        u()


def _host_inputs(x, W_attn, b_attn, lora_A, lora_B, W_proj, b_proj):
    """Build the 8 per-core input maps (all numpy, bf16 for GEMM operands)."""
    bf = ml_dtypes.bfloat16
    x = np.asarray(x, np.float32)
    W_attn = np.asarray(W_attn, np.float32)
    b_attn = np.asarray(b_attn, np.float32)
    lora_A = np.asarray(lora_A, np.float32)
    lora_B = np.asarray(lora_B, np.float32)
    W_proj = np.asarray(W_proj, np.float32)

    Wq = W_attn[:, :NX] + SCALING * (lora_A[:R].T @ lora_B[:NX].T)
    Wk = W_attn[:, NX:2 * NX]
    Wv = W_attn[:, 2 * NX:] + SCALING * (lora_A[R:].T @ lora_B[NX:].T)

    # causal diagonal masks: keep when f >= 128*d + p
    f = np.arange(ST)[None, :]
    p = np.arange(P)[:, None]
    msk = np.ascontiguousarray(np.stack(
        [(f >= (P * d + p)) for d in range(4)]).transpose(1, 0, 2)).astype(bf)

    in_maps = []
    for core in range(NCORES):
        b, g = core // 4, core % 4
        cs = CPW * g
        xt = np.ascontiguousarray(
            x[b].T.reshape(DC, P, NSC, P).transpose(2, 0, 1, 3)).astype(bf)
        wqk = np.concatenate([Wq[:, cs:cs + CPW], Wk[:, cs:cs + CPW]], axis=1)
        wqk = np.ascontiguousarray(
            wqk.reshape(DC, P, 2 * CPW).transpose(1, 0, 2)).astype(bf)
        wv = np.ascontiguousarray(
            Wv[:, cs:cs + CPW].reshape(DC, P, CPW).transpose(1, 0, 2)).astype(bf)
        wp = np.ascontiguousarray(
            W_proj[cs:cs + CPW].reshape(2, P, NX).transpose(1, 0, 2)).astype(bf)
        bqk = np.stack([b_attn[cs:cs + P], b_attn[cs + P:cs + CPW],
                        b_attn[NX + cs:NX + cs + P],
                        b_attn[NX + cs + P:NX + cs + CPW]], axis=1)
        bqk = np.ascontiguousarray(bqk, np.float32)
        bv = b_attn[2 * NX + cs:2 * NX + cs + CPW].reshape(1, CPW).astype(bf)
        in_maps.append({"xt": xt, "wqk": wqk, "wv": wv, "wp": wp,
                        "bqk": bqk, "bv": bv, "msk": msk})
    return in_maps


def _assemble(results, b_attn, b_proj):
    b_proj = np.asarray(b_proj, np.float32)
    out = np.zeros((B, S, NX), np.float32)
    present = np.zeros((2, B, NH, S, HD), np.float32)
    for core in range(NCORES):
        b, g = core // 4, core % 4
        r = results[core]
        out[b] += r["part"].reshape(S, NX).astype(np.float32)
        kt = r["kt_out"].reshape(CPW, S).astype(np.float32)
        v = r["v_out"].reshape(S, VAUG_W).astype(np.float32)
        for hl in range(HPC):
            present[0, b, HPC * g + hl] = kt[HD * hl:HD * (hl + 1), :].T
            cs = V_COL_START[hl]
            present[1, b, HPC * g + hl] = v[:, cs:cs + HD]
    out += b_proj[None, None, :]
    return out, present


def get_graph():
    if "nc" not in _CACHED:
        _CACHED["nc"] = _build_graph()
    return _CACHED["nc"]


def kernel(x, W_attn, b_attn, lora_A, lora_B, W_proj, b_proj, **extra):
    nc = get_graph()
    in_maps = _host_inputs(x, W_attn, b_attn, lora_A, lora_B, W_proj, b_proj)
    res = run_bass_kernel_spmd(nc, in_maps, core_ids=list(range(NCORES)))
    return _assemble(res.results, b_attn, b_proj)
